# revision 1
# baseline (speedup 1.0000x reference)
import numpy as np

# Problem constants (hardcoded per spec: nn_GaussianMixture, N=16384, K=128, D=32)
N, K, D = 16384, 128, 32
TWO_PI = 2.0 * np.pi
N_CORES = 8


def _score_shard(x, sigma, phi, mu, L_eig, Q):
    """GMM noise-conditioned score for one shard of samples.

    x: (Ns, D), sigma: (Ns,), phi: (K,), mu: (K, D), L_eig: (K, D), Q: (K, D, D)
    Returns (Ns, D).
    """
    import jax.numpy as jnp
    L = L_eig[None, :, :] + (sigma ** 2)[:, None, None]          # (Ns, K, D)
    y = mu[None, :, :] - x[:, None, :]                           # (Ns, K, D)
    # u = Q^T y  (per component): u_{nkj} = sum_l Q_{klj} y_{nkl}
    u = jnp.einsum('klj,nkl->nkj', Q, y)                         # (Ns, K, D)
    t = u / L                                                    # (Ns, K, D)
    z = jnp.einsum('kij,nkj->nki', Q, t)                         # (Ns, K, D)
    quad = jnp.sum(u * t, axis=-1)                               # (Ns, K)
    d = jnp.prod(L, axis=-1)                                     # (Ns, K)
    c = phi[None, :] / jnp.sqrt((TWO_PI ** D) * d)               # (Ns, K)
    w = c * jnp.exp(-0.5 * quad)                                 # (Ns, K)
    num = jnp.einsum('nk,nki->ni', w, z)                         # (Ns, D)
    den = jnp.sum(w, axis=-1)[:, None]                           # (Ns, 1)
    return num / den


def _kernel_jax(x, sigma, phi, mu, L_eig, Q):
    import jax
    devs = jax.devices()
    ndev = N_CORES if len(devs) >= N_CORES else 1
    ns = x.shape[0] // ndev
    xs = x.reshape(ndev, ns, D)
    ss = sigma.reshape(ndev, ns)
    f = jax.pmap(_score_shard, in_axes=(0, 0, None, None, None, None),
                 devices=devs[:ndev])
    out = f(xs, ss, phi, mu, L_eig, Q)
    return np.asarray(out).reshape(x.shape[0], D).astype(np.float32)


def _kernel_np(x, sigma, phi, mu, L_eig, Q):
    out = np.empty_like(x)
    chunk = 1024
    for s in range(0, x.shape[0], chunk):
        xe = x[s:s + chunk]
        se = sigma[s:s + chunk]
        L = L_eig[None] + (se ** 2)[:, None, None]
        y = mu[None] - xe[:, None, :]
        u = np.einsum('klj,nkl->nkj', Q, y)
        t = u / L
        z = np.einsum('kij,nkj->nki', Q, t)
        quad = np.sum(u * t, axis=-1)
        d = np.prod(L, axis=-1)
        c = phi[None, :] / np.sqrt((TWO_PI ** D) * d)
        w = c * np.exp(-0.5 * quad)
        num = np.einsum('nk,nki->ni', w, z)
        out[s:s + chunk] = num / np.sum(w, axis=-1)[:, None]
    return out.astype(np.float32)


def kernel(x, sigma, phi, mu, L_eig, Q):
    x = np.asarray(x, dtype=np.float32)
    sigma = np.asarray(sigma, dtype=np.float32)
    phi = np.asarray(phi, dtype=np.float32)
    mu = np.asarray(mu, dtype=np.float32)
    L_eig = np.asarray(L_eig, dtype=np.float32)
    Q = np.asarray(Q, dtype=np.float32)
    try:
        return _kernel_jax(x, sigma, phi, mu, L_eig, Q)
    except Exception:
        return _kernel_np(x, sigma, phi, mu, L_eig, Q)

